# revision 27
# baseline (speedup 1.0000x reference)
"""Trainium2 Bass kernel for nn_DiceLossLayer.

Data-parallel over batch: 64 polygons/masks split as 8 batches on each of 8
NeuronCores. Each core rasterizes its polygons with a sort-free scanline
algorithm and reduces dice statistics; host averages the per-core results.

Math (replaces the reference's sort-based scanline pairing):
  For scanline y, let xint_i be the x-intercepts of the crossing edges and
  A(y,x) = #{i : xint_i < x}. The even-odd filled mask is exactly
      mask(y,x) = (A(y,x) mod 2 == 1) OR (A(y,x+1) > A(y,x))
  (the OR term reproduces the reference's floor()-based pair endpoints).
  A is accumulated per-edge with fused compare+add ops, split across the
  DVE and Pool engines; dice sums use fused reduce ops and one PE matmul
  for the final partition reduction.
"""

import os

import numpy as np

os.environ.setdefault("JAX_PLATFORMS", "")

import concourse.bacc as bacc
import concourse.bass as bass
import concourse.tile as tile
from concourse import mybir
from concourse.bass_utils import run_bass_kernel_spmd

F32 = mybir.dt.float32
ALU = mybir.AluOpType

N_CORES = 8
B = 8           # batches per core
NV = 128        # polygon vertices (= edges)
GRID = 256
XN = GRID + 1   # A evaluated at x = 0..256
SMOOTH = 1e-6

# edge-lane split across engines
EDGES_DVE = 62
EDGES_POOL = 11


def _q_thresh() -> float:
    # largest f32 d with fl(d * 255f) <= 127f
    d = np.float32(127.0) / np.float32(255.0)
    one = np.float32(1.0)
    while np.nextafter(d, one) * np.float32(255.0) <= np.float32(127.0):
        d = np.nextafter(d, one)
    return float(d)


Q_THRESH = _q_thresh()

_CACHE = {}


def _emit(ctx, tc, pts_d, dmap_d, ident_d, stats_d):
    nc = tc.nc

    setup = ctx.enter_context(tc.tile_pool(name="setup", bufs=1))
    dmp = ctx.enter_context(tc.tile_pool(name="dmp", bufs=3))
    geo = ctx.enter_context(tc.tile_pool(name="geo", bufs=1))
    accp = ctx.enter_context(tc.tile_pool(name="accp", bufs=3))
    post = ctx.enter_context(tc.tile_pool(name="post", bufs=2))
    psum = ctx.enter_context(tc.tile_pool(name="psum", bufs=2, space="PSUM"))
    psbc = ctx.enter_context(tc.tile_pool(name="psbc", bufs=4, space="PSUM"))

    # ---------------- one-time setup ----------------
    sb_pts = setup.tile([NV, 2 * B], F32)
    nc.sync.dma_start(sb_pts[:], pts_d[:])
    sb_ident = setup.tile([128, 128], F32)
    nc.sync.dma_start(sb_ident[:], ident_d[:])

    # pc = clip(pts*255, 0, 255)
    sb_pc = setup.tile([NV, 2 * B], F32)
    nc.vector.tensor_scalar(sb_pc[:], sb_pts[:], 255.0, 255.0, ALU.mult, ALU.min)
    nc.vector.tensor_scalar(sb_pc[:], sb_pc[:], 0.0, None, ALU.max)

    # pj = roll(pc, 1, axis=partition)
    sb_pj = setup.tile([NV, 2 * B], F32)
    nc.sync.dma_start(sb_pj[1:NV, :], sb_pc[0 : NV - 1, :])
    nc.sync.dma_start(sb_pj[0:1, :], sb_pc[NV - 1 : NV, :])

    pc3 = sb_pc.rearrange("p (b c) -> p b c", c=2)
    pj3 = sb_pj.rearrange("p (b c) -> p b c", c=2)
    pix = pc3[:, :, 0:1]
    piy = pc3[:, :, 1:2]
    pjx = pj3[:, :, 0:1]
    pjy = pj3[:, :, 1:2]

    # params packed [128, b*4 + {piy, pjy, pix, negslope}]
    sb_prm = setup.tile([NV, 4 * B], F32)
    prm3 = sb_prm.rearrange("p (b k) -> p b k", k=4)
    nc.vector.tensor_copy(prm3[:, :, 0:1], piy)
    nc.vector.tensor_copy(prm3[:, :, 1:2], pjy)
    nc.vector.tensor_copy(prm3[:, :, 2:3], pix)

    sb_d = setup.tile([NV, B], F32)
    d3 = sb_d.rearrange("p (b k) -> p b k", k=1)
    nc.vector.tensor_tensor(d3, pjy, piy, ALU.subtract)
    sb_z = setup.tile([NV, B], F32)
    nc.vector.tensor_scalar(sb_z[:], sb_d[:], 0.0, None, ALU.is_equal)
    nc.vector.tensor_tensor(sb_d[:], sb_d[:], sb_z[:], ALU.add)
    sb_rcp = setup.tile([NV, B], F32)
    nc.vector.reciprocal(sb_rcp[:], sb_d[:])
    sb_t = setup.tile([NV, B], F32)
    t3 = sb_t.rearrange("p (b k) -> p b k", k=1)
    nc.vector.tensor_tensor(t3, pix, pjx, ALU.subtract)
    rcp3 = sb_rcp.rearrange("p (b k) -> p b k", k=1)
    nc.vector.tensor_tensor(prm3[:, :, 3:4], t3, rcp3, ALU.mult)
    nc.vector.tensor_scalar(prm3[:, :, 3:4], prm3[:, :, 3:4], 1e20, -1e20,
                            ALU.min, ALU.max)

    # transpose params, flatten the 32 rows onto one partition, then
    # DMA-broadcast that row to all 128 partitions (partition-step-0 read)
    ps_prmT = psum.tile([4 * B, NV], F32)
    nc.tensor.transpose(ps_prmT[:], sb_prm[:], sb_ident[:])
    sb_prmT = setup.tile([4 * B, NV], F32)
    nc.vector.tensor_copy(sb_prmT[:], ps_prmT[:])

    prmrow_d = nc.dram_tensor("prmrow", [4 * B * NV], F32)
    nc.sync.dma_start(prmrow_d[:], sb_prmT[:])

    row_ap = prmrow_d[:]
    bcast_src = bass.AP(
        tensor=row_ap.tensor,
        offset=row_ap.offset,
        ap=[[0, 128]] + list(row_ap.ap),
    )
    sb_B = setup.tile([128, 4 * B * NV], F32)
    nc.sync.dma_start(sb_B[:], bcast_src)

    # iotas
    sb_ix = setup.tile([128, XN], F32)
    nc.gpsimd.iota(sb_ix[:], pattern=[[1, XN]], base=0, channel_multiplier=0,
                   allow_small_or_imprecise_dtypes=True)
    sb_iy = setup.tile([128, 2], F32)
    nc.gpsimd.iota(sb_iy[:], pattern=[[128, 2]], base=0, channel_multiplier=1,
                   allow_small_or_imprecise_dtypes=True)

    sb_onescol = setup.tile([128, 1], F32)
    nc.vector.memset(sb_onescol[:], 1.0)

    sb_stats = setup.tile([128, 6 * B], F32)

    def _pk(t, k):
        # all batches' param-k blocks, as [128, B*NV] with batch-major cols:
        # layout of sb_B is (b, k, i); rearrange to pick k across batches
        v = t.rearrange("p (b k i) -> p b k i", b=B, k=4)
        return v[:, :, k, :]

    # ---------------- batched geometry (all batches, per y-chunk) ----------------
    # xm_all[c][:, b*NV + i] = masked x-intercept of edge i, batch b, rows of
    # chunk c;  negxm_all holds the ACT sign-lane thresholds
    # theta = rnd(xm+0.5)-0.5 (half-integers: [x > theta] == [x > xm]).
    W = B * NV
    xm_all = []
    negxm_all = []
    for c in range(2):
        iy = sb_iy[:, c : c + 1]
        c1 = geo.tile([128, W], F32, tag="g_c1")
        nc.vector.tensor_scalar(c1[:], _pk(sb_B, 0), iy, None, ALU.is_lt)
        c2 = geo.tile([128, W], F32, tag="g_c2")
        nc.vector.tensor_scalar(c2[:], _pk(sb_B, 1), iy, None, ALU.is_lt)
        cross = geo.tile([128, W], F32, tag="g_cross")
        nc.vector.tensor_tensor(cross[:], c1[:], c2[:], ALU.not_equal)
        t1 = geo.tile([128, W], F32, tag="g_t1")
        nc.vector.scalar_tensor_tensor(t1[:], _pk(sb_B, 0), iy, _pk(sb_B, 3),
                                       ALU.subtract, ALU.mult)
        xint = geo.tile([128, W], F32, tag="g_xint")
        nc.gpsimd.tensor_tensor(xint[:], t1[:], _pk(sb_B, 2), ALU.add)
        t2 = geo.tile([128, W], F32, tag="g_t2")
        nc.vector.scalar_tensor_tensor(t2[:], xint[:], -300.0, cross[:],
                                       ALU.add, ALU.mult)
        xm = geo.tile([128, W], F32, tag=f"g_xm{c}", name=f"g_xm{c}")
        nc.gpsimd.tensor_scalar(xm[:], t2[:], 300.0, None, ALU.add)
        w1 = geo.tile([128, W], F32, tag="g_w1")
        nc.gpsimd.tensor_scalar(w1[:], xm[:], 0.5, 8388608.0, ALU.add, ALU.add)
        r1 = geo.tile([128, W], F32, tag="g_r1")
        nc.gpsimd.tensor_scalar(r1[:], w1[:], -8388608.0, None, ALU.add)
        negxm = geo.tile([128, W], F32, tag=f"g_negxm{c}", name=f"g_negxm{c}")
        nc.gpsimd.tensor_scalar(negxm[:], r1[:], -1.0, 0.5, ALU.mult, ALU.add)
        xm_all.append(xm)
        negxm_all.append(negxm)

    # ---------------- main loop ----------------
    for b in range(B):
        for c in range(2):
            xm = xm_all[c]
            negxm = negxm_all[c]

            sb_dm = dmp.tile([128, GRID], F32, tag="dm")
            nc.sync.dma_start(sb_dm[:], dmap_d[b, c * 128 : (c + 1) * 128, :])

            # per-edge accumulation of A(y, x) = sum_e [x > xm_e], split:
            #  - DVE: fused (ix > xm) + acc  (comparisons are DVE-only)
            #  - ACT: sign(ix - xm - 0.5) rows; Pool accumulates them.
            #    sum_e sign = 2*count - n  ->  count = (sum + n)/2; masked
            #    edges (xm=300) give sign=-1 everywhere -> count 0. exact.
            ND, NA, NP = 5, 4, 2
            off = b * NV
            accD = [accp.tile([128, XN], F32, tag=f"accD{k}", name=f"accD{k}")
                    for k in range(ND)]
            accA = [accp.tile([128, XN], F32, tag=f"accA{k}", name=f"accA{k}")
                    for k in range(NA)]
            accP = [accp.tile([128, XN], F32, tag=f"accP{k}", name=f"accP{k}")
                    for k in range(NP)]
            sgn = [accp.tile([128, XN], F32, tag=f"sgn{k}", name=f"sgn{k}")
                   for k in range(8)]
            clp = [accp.tile([128, XN], F32, tag=f"clp{k}", name=f"clp{k}")
                   for k in range(4)]
            # lane split: DVE fused compare-add / ACT sign rows + Pool adds /
            # Pool clamp edges (theta half-integer makes clamp exact 0/1)
            n_dve, n_act, n_pool = EDGES_DVE, NV - EDGES_DVE - EDGES_POOL, EDGES_POOL
            for e in range(n_dve):
                col = xm[:, off + e : off + e + 1]
                acc = accD[e % ND]
                if e < ND:
                    nc.vector.tensor_scalar(acc[:], sb_ix[:], col, None, ALU.is_gt)
                else:
                    nc.vector.scalar_tensor_tensor(acc[:], sb_ix[:], col, acc[:],
                                                   ALU.is_gt, ALU.add)
            for j in range(n_act):
                e = n_dve + j
                bias = negxm[:, off + e : off + e + 1]
                k = j % NA
                if j < NA:
                    nc.scalar.activation(accA[k][:], sb_ix[:],
                                         mybir.ActivationFunctionType.Sign,
                                         bias=bias, scale=1.0)
                else:
                    s = sgn[j % 8]
                    nc.scalar.activation(s[:], sb_ix[:],
                                         mybir.ActivationFunctionType.Sign,
                                         bias=bias, scale=1.0)
                    nc.gpsimd.tensor_tensor(accA[k][:], accA[k][:], s[:], ALU.add)
            # Pool clamp lane: c = min(max(ix - theta, 0), 1) with theta the
            # half-integer -negxm => c == [x > xm] exactly
            for j in range(n_pool):
                e = n_dve + n_act + j
                col = negxm[:, off + e : off + e + 1]
                s = clp[j % 4]
                nc.gpsimd.tensor_scalar(s[:], sb_ix[:], col, 0.5, ALU.add, ALU.add)
                k = j % NP
                if j < NP:
                    nc.gpsimd.tensor_scalar(accP[k][:], s[:], 0.0, 1.0, ALU.max,
                                            ALU.min)
                else:
                    s2 = clp[j % 4]
                    nc.gpsimd.tensor_scalar(s2[:], s[:], 0.0, 1.0, ALU.max, ALU.min)
                    nc.gpsimd.tensor_tensor(accP[k][:], accP[k][:], s2[:], ALU.add)

            cm1 = accp.tile([128, XN], F32, tag="cm1")
            nc.gpsimd.tensor_tensor(cm1[:], accD[0][:], accD[1][:], ALU.add)
            cm2 = accp.tile([128, XN], F32, tag="cm2")
            nc.gpsimd.tensor_tensor(cm2[:], accD[2][:], accD[3][:], ALU.add)
            cm2b = accp.tile([128, XN], F32, tag="cm2b")
            nc.gpsimd.tensor_tensor(cm2b[:], cm2[:], accD[4][:], ALU.add)
            cm3 = accp.tile([128, XN], F32, tag="cm3")
            nc.gpsimd.tensor_tensor(cm3[:], accA[0][:], accA[1][:], ALU.add)
            cm4 = accp.tile([128, XN], F32, tag="cm4")
            nc.gpsimd.tensor_tensor(cm4[:], accA[2][:], accA[3][:], ALU.add)
            cm5 = accp.tile([128, XN], F32, tag="cm5")
            nc.gpsimd.tensor_tensor(cm5[:], cm1[:], cm2b[:], ALU.add)
            cm6 = accp.tile([128, XN], F32, tag="cm6")
            nc.gpsimd.tensor_tensor(cm6[:], cm3[:], cm4[:], ALU.add)
            cm7 = accp.tile([128, XN], F32, tag="cm7")
            nc.gpsimd.tensor_scalar(cm7[:], cm6[:], float(NV - EDGES_DVE - EDGES_POOL),
                                    0.5, ALU.add, ALU.mult)
            cm8 = accp.tile([128, XN], F32, tag="cm8")
            nc.gpsimd.tensor_tensor(cm8[:], accP[0][:], accP[1][:], ALU.add)
            cm9 = accp.tile([128, XN], F32, tag="cm9")
            nc.gpsimd.tensor_tensor(cm9[:], cm5[:], cm8[:], ALU.add)
            accT = accp.tile([128, XN], F32, tag="accT")
            nc.gpsimd.tensor_tensor(accT[:], cm9[:], cm7[:], ALU.add)

            # mask = (A mod 2) | (A(x+1) > A(x));   dice partial sums
            # parity(A) exactly in f32: r = rnd_half_even(A/2) via the 2^23
            # trick, d = A - 2r in {0, +-1}, par = d^2
            TWO23 = 8388608.0
            u = post.tile([128, GRID], F32, tag="paru")
            nc.gpsimd.tensor_scalar(u[:], accT[:, 0:GRID], 0.5, TWO23,
                                    ALU.mult, ALU.add)
            r = post.tile([128, GRID], F32, tag="parr")
            nc.gpsimd.tensor_scalar(r[:], u[:], -TWO23, None, ALU.add)
            dpar = post.tile([128, GRID], F32, tag="pard")
            nc.vector.scalar_tensor_tensor(dpar[:], r[:], -2.0, accT[:, 0:GRID],
                                           ALU.mult, ALU.add)
            par = post.tile([128, GRID], F32, tag="par")
            nc.gpsimd.tensor_tensor(par[:], dpar[:], dpar[:], ALU.mult)
            bnd = post.tile([128, GRID], F32, tag="bnd")
            nc.vector.tensor_tensor(bnd[:], accT[:, 1:XN], accT[:, 0:GRID], ALU.is_gt)
            col0 = 6 * b + 3 * c
            mask = post.tile([128, GRID], F32, tag="mask")
            nc.vector.scalar_tensor_tensor(
                mask[:], par[:], 0.0, bnd[:], ALU.add, ALU.max,
                accum_out=sb_stats[:, col0 : col0 + 1])

            # q = (dmap*255 <= 127), rewritten as dmap <= Q_THRESH (exact:
            # x -> fl(x*255) is monotone, Q_THRESH is the largest f32 passing).
            # op1 here is the accumulator's reduce op (sum -> Q stat).
            q = post.tile([128, GRID], F32, tag="q")
            nc.vector.tensor_scalar(q[:], sb_dm[:], Q_THRESH, None, ALU.is_le,
                                    ALU.add,
                                    accum_out=sb_stats[:, col0 + 2 : col0 + 3])

            prod = post.tile([128, GRID], F32, tag="prod")
            nc.vector.scalar_tensor_tensor(
                prod[:], mask[:], 0.0, q[:], ALU.add, ALU.mult,
                accum_out=sb_stats[:, col0 + 1 : col0 + 2])

    # ---------------- final reduction over partitions ----------------
    ps_stats = psum.tile([6 * B, 1], F32)
    nc.tensor.matmul(ps_stats[:], sb_stats[:], sb_onescol[:],
                     start=True, stop=True)
    sb_final = setup.tile([6 * B, 1], F32)
    nc.vector.tensor_copy(sb_final[:], ps_stats[:])
    nc.sync.dma_start(stats_d[:], sb_final[:])


def _build():
    if "nc" in _CACHE:
        return _CACHE["nc"]
    nc = bacc.Bacc(None, target_bir_lowering=False, debug=False)
    pts_d = nc.dram_tensor("pts", [NV, 2 * B], F32, kind="ExternalInput")
    dmap_d = nc.dram_tensor("dmap", [B, GRID, GRID], F32, kind="ExternalInput")
    ident_d = nc.dram_tensor("ident", [128, 128], F32, kind="ExternalInput")
    stats_d = nc.dram_tensor("stats", [6 * B, 1], F32, kind="ExternalOutput")
    from contextlib import ExitStack

    with tile.TileContext(nc) as tc:
        with ExitStack() as ctx:
            _emit(ctx, tc, pts_d, dmap_d, ident_d, stats_d)
    if hasattr(nc, "compile"):
        nc.compile()
    else:
        nc.finalize()
    _CACHE["nc"] = nc
    return nc


def kernel(points: np.ndarray, dmap: np.ndarray) -> np.ndarray:
    pts = np.asarray(points, dtype=np.float32).reshape(64, NV, 2)
    dm = np.asarray(dmap, dtype=np.float32).reshape(64, GRID, GRID)
    ident = np.eye(128, dtype=np.float32)

    in_maps = []
    for r in range(N_CORES):
        sl = slice(r * B, (r + 1) * B)
        pts_r = np.ascontiguousarray(pts[sl].transpose(1, 0, 2).reshape(NV, 2 * B))
        in_maps.append({
            "pts": pts_r,
            "dmap": np.ascontiguousarray(dm[sl]),
            "ident": ident,
        })

    nc = _build()
    res = run_bass_kernel_spmd(nc, in_maps, core_ids=list(range(N_CORES)))

    dices = []
    for r in range(N_CORES):
        s = np.asarray(res.results[r]["stats"], dtype=np.float32).reshape(B, 2, 3)
        s = s.sum(axis=1)  # combine the two row-chunks
        T, I, Q = s[:, 0], s[:, 1], s[:, 2]
        dices.append((np.float32(2.0) * I + np.float32(SMOOTH))
                     / (T + Q + np.float32(SMOOTH)))
    dices = np.concatenate(dices).astype(np.float32)
    return np.float32(np.mean(np.float32(1.0) - dices))


# revision 31
# speedup vs baseline: 1.0272x; 1.0272x over previous
"""Trainium2 Bass kernel for nn_DiceLossLayer.

Data-parallel over batch: 64 polygons/masks split as 8 batches on each of 8
NeuronCores. Each core rasterizes its polygons with a sort-free scanline
algorithm and reduces dice statistics; host averages the per-core results.

Math (replaces the reference's sort-based scanline pairing):
  For scanline y, let xint_i be the x-intercepts of the crossing edges and
  A(y,x) = #{i : xint_i < x}. The even-odd filled mask is exactly
      mask(y,x) = (A(y,x) mod 2 == 1) OR (A(y,x+1) > A(y,x))
  (the OR term reproduces the reference's floor()-based pair endpoints).
  A is accumulated per-edge with fused compare+add ops, split across the
  DVE and Pool engines; dice sums use fused reduce ops and one PE matmul
  for the final partition reduction.
"""

import os

import numpy as np

os.environ.setdefault("JAX_PLATFORMS", "")

import concourse.bacc as bacc
import concourse.bass as bass
import concourse.tile as tile
from concourse import mybir
from concourse.bass_utils import run_bass_kernel_spmd

F32 = mybir.dt.float32
ALU = mybir.AluOpType

N_CORES = 8
B = 8           # batches per core
NV = 128        # polygon vertices (= edges)
GRID = 256
XN = GRID + 1   # A evaluated at x = 0..256
SMOOTH = 1e-6

# edge-lane split across engines
EDGES_DVE = 63
EDGES_POOL = 13


def _q_thresh() -> float:
    # largest f32 d with fl(d * 255f) <= 127f
    d = np.float32(127.0) / np.float32(255.0)
    one = np.float32(1.0)
    while np.nextafter(d, one) * np.float32(255.0) <= np.float32(127.0):
        d = np.nextafter(d, one)
    return float(d)


Q_THRESH = _q_thresh()

_CACHE = {}


def _emit(ctx, tc, pts_d, dmap_d, ident_d, stats_d):
    nc = tc.nc

    setup = ctx.enter_context(tc.tile_pool(name="setup", bufs=1))
    dmp = ctx.enter_context(tc.tile_pool(name="dmp", bufs=3))
    geo = ctx.enter_context(tc.tile_pool(name="geo", bufs=1))
    accp = ctx.enter_context(tc.tile_pool(name="accp", bufs=3))
    post = ctx.enter_context(tc.tile_pool(name="post", bufs=2))
    psum = ctx.enter_context(tc.tile_pool(name="psum", bufs=2, space="PSUM"))

    # ---------------- one-time setup ----------------
    sb_pts = setup.tile([NV, 2 * B], F32)
    nc.sync.dma_start(sb_pts[:], pts_d[:])
    sb_ident = setup.tile([128, 128], F32)
    nc.sync.dma_start(sb_ident[:], ident_d[:])

    # pc = clip(pts*255, 0, 255)
    sb_pc = setup.tile([NV, 2 * B], F32)
    nc.vector.tensor_scalar(sb_pc[:], sb_pts[:], 255.0, 255.0, ALU.mult, ALU.min)
    nc.vector.tensor_scalar(sb_pc[:], sb_pc[:], 0.0, None, ALU.max)

    # pj = roll(pc, 1, axis=partition)
    sb_pj = setup.tile([NV, 2 * B], F32)
    nc.sync.dma_start(sb_pj[1:NV, :], sb_pc[0 : NV - 1, :])
    nc.sync.dma_start(sb_pj[0:1, :], sb_pc[NV - 1 : NV, :])

    pc3 = sb_pc.rearrange("p (b c) -> p b c", c=2)
    pj3 = sb_pj.rearrange("p (b c) -> p b c", c=2)
    pix = pc3[:, :, 0:1]
    piy = pc3[:, :, 1:2]
    pjx = pj3[:, :, 0:1]
    pjy = pj3[:, :, 1:2]

    # params packed [128, b*4 + {piy, pjy, pix, negslope}]
    sb_prm = setup.tile([NV, 4 * B], F32)
    prm3 = sb_prm.rearrange("p (b k) -> p b k", k=4)
    nc.vector.tensor_copy(prm3[:, :, 0:1], piy)
    nc.vector.tensor_copy(prm3[:, :, 1:2], pjy)
    nc.vector.tensor_copy(prm3[:, :, 2:3], pix)

    sb_d = setup.tile([NV, B], F32)
    d3 = sb_d.rearrange("p (b k) -> p b k", k=1)
    nc.vector.tensor_tensor(d3, pjy, piy, ALU.subtract)
    sb_z = setup.tile([NV, B], F32)
    nc.vector.tensor_scalar(sb_z[:], sb_d[:], 0.0, None, ALU.is_equal)
    nc.vector.tensor_tensor(sb_d[:], sb_d[:], sb_z[:], ALU.add)
    sb_rcp = setup.tile([NV, B], F32)
    nc.vector.reciprocal(sb_rcp[:], sb_d[:])
    sb_t = setup.tile([NV, B], F32)
    t3 = sb_t.rearrange("p (b k) -> p b k", k=1)
    nc.vector.tensor_tensor(t3, pix, pjx, ALU.subtract)
    rcp3 = sb_rcp.rearrange("p (b k) -> p b k", k=1)
    nc.vector.tensor_tensor(prm3[:, :, 3:4], t3, rcp3, ALU.mult)
    nc.vector.tensor_scalar(prm3[:, :, 3:4], prm3[:, :, 3:4], 1e20, -1e20,
                            ALU.min, ALU.max)

    # transpose params, flatten the 32 rows onto one partition, then
    # DMA-broadcast that row to all 128 partitions (partition-step-0 read)
    ps_prmT = psum.tile([4 * B, NV], F32)
    nc.tensor.transpose(ps_prmT[:], sb_prm[:], sb_ident[:])
    sb_prmT = setup.tile([4 * B, NV], F32)
    nc.vector.tensor_copy(sb_prmT[:], ps_prmT[:])

    prmrow_d = nc.dram_tensor("prmrow", [4 * B * NV], F32)
    nc.sync.dma_start(prmrow_d[:], sb_prmT[:])

    row_ap = prmrow_d[:]
    bcast_src = bass.AP(
        tensor=row_ap.tensor,
        offset=row_ap.offset,
        ap=[[0, 128]] + list(row_ap.ap),
    )
    sb_B = setup.tile([128, 4 * B * NV], F32)
    nc.sync.dma_start(sb_B[:], bcast_src)

    # iotas
    sb_ix = setup.tile([128, XN], F32)
    nc.gpsimd.iota(sb_ix[:], pattern=[[1, XN]], base=0, channel_multiplier=0,
                   allow_small_or_imprecise_dtypes=True)
    sb_iy = setup.tile([128, 2], F32)
    nc.gpsimd.iota(sb_iy[:], pattern=[[128, 2]], base=0, channel_multiplier=1,
                   allow_small_or_imprecise_dtypes=True)

    sb_onescol = setup.tile([128, 1], F32)
    nc.vector.memset(sb_onescol[:], 1.0)

    sb_stats = setup.tile([128, 6 * B], F32)

    def _pk(t, k):
        # all batches' param-k blocks, as [128, B*NV] with batch-major cols:
        # layout of sb_B is (b, k, i); rearrange to pick k across batches
        v = t.rearrange("p (b k i) -> p b k i", b=B, k=4)
        return v[:, :, k, :]

    # ---------------- batched geometry (all batches, per y-chunk) ----------------
    # xm_all[c][:, b*NV + i] = masked x-intercept of edge i, batch b, rows of
    # chunk c;  negxm_all holds the ACT sign-lane thresholds
    # theta = rnd(xm+0.5)-0.5 (half-integers: [x > theta] == [x > xm]).
    W = B * NV
    xm_all = []
    negxm_all = []
    for c in range(2):
        iy = sb_iy[:, c : c + 1]
        c1 = geo.tile([128, W], F32, tag="g_c1")
        nc.vector.tensor_scalar(c1[:], _pk(sb_B, 0), iy, None, ALU.is_lt)
        c2 = geo.tile([128, W], F32, tag="g_c2")
        nc.vector.tensor_scalar(c2[:], _pk(sb_B, 1), iy, None, ALU.is_lt)
        cross = geo.tile([128, W], F32, tag="g_cross")
        nc.vector.tensor_tensor(cross[:], c1[:], c2[:], ALU.not_equal)
        t1 = geo.tile([128, W], F32, tag="g_t1")
        nc.vector.scalar_tensor_tensor(t1[:], _pk(sb_B, 0), iy, _pk(sb_B, 3),
                                       ALU.subtract, ALU.mult)
        xint = geo.tile([128, W], F32, tag="g_xint")
        nc.gpsimd.tensor_tensor(xint[:], t1[:], _pk(sb_B, 2), ALU.add)
        t2 = geo.tile([128, W], F32, tag="g_t2")
        nc.vector.scalar_tensor_tensor(t2[:], xint[:], -300.0, cross[:],
                                       ALU.add, ALU.mult)
        xm = geo.tile([128, W], F32, tag=f"g_xm{c}", name=f"g_xm{c}")
        nc.gpsimd.tensor_scalar(xm[:], t2[:], 300.0, None, ALU.add)
        w1 = geo.tile([128, W], F32, tag="g_w1")
        nc.gpsimd.tensor_scalar(w1[:], xm[:], 0.5, 8388608.0, ALU.add, ALU.add)
        r1 = geo.tile([128, W], F32, tag="g_r1")
        nc.gpsimd.tensor_scalar(r1[:], w1[:], -8388608.0, None, ALU.add)
        negxm = geo.tile([128, W], F32, tag=f"g_negxm{c}", name=f"g_negxm{c}")
        nc.gpsimd.tensor_scalar(negxm[:], r1[:], -1.0, 0.5, ALU.mult, ALU.add)
        xm_all.append(xm)
        negxm_all.append(negxm)

    # ---------------- main loop ----------------
    for b in range(B):
        for c in range(2):
            xm = xm_all[c]
            negxm = negxm_all[c]

            sb_dm = dmp.tile([128, GRID], F32, tag="dm")
            nc.sync.dma_start(sb_dm[:], dmap_d[b, c * 128 : (c + 1) * 128, :])

            # per-edge accumulation of A(y, x) = sum_e [x > xm_e], split:
            #  - DVE: fused (ix > xm) + acc  (comparisons are DVE-only)
            #  - ACT: sign(ix - xm - 0.5) rows; Pool accumulates them.
            #    sum_e sign = 2*count - n  ->  count = (sum + n)/2; masked
            #    edges (xm=300) give sign=-1 everywhere -> count 0. exact.
            ND, NA, NP = 5, 4, 2
            off = b * NV
            accD = [accp.tile([128, XN], F32, tag=f"accD{k}", name=f"accD{k}")
                    for k in range(ND)]
            accA = [accp.tile([128, XN], F32, tag=f"accA{k}", name=f"accA{k}")
                    for k in range(NA)]
            accP = [accp.tile([128, XN], F32, tag=f"accP{k}", name=f"accP{k}")
                    for k in range(NP)]
            sgn = [accp.tile([128, XN], F32, tag=f"sgn{k}", name=f"sgn{k}")
                   for k in range(8)]
            clp = [accp.tile([128, XN], F32, tag=f"clp{k}", name=f"clp{k}")
                   for k in range(4)]
            # lane split: DVE fused compare-add / ACT sign rows + Pool adds /
            # Pool clamp edges (theta half-integer makes clamp exact 0/1)
            n_dve, n_act, n_pool = EDGES_DVE, NV - EDGES_DVE - EDGES_POOL, EDGES_POOL
            for e in range(n_dve):
                col = xm[:, off + e : off + e + 1]
                acc = accD[e % ND]
                if e < ND:
                    nc.vector.tensor_scalar(acc[:], sb_ix[:], col, None, ALU.is_gt)
                else:
                    nc.vector.scalar_tensor_tensor(acc[:], sb_ix[:], col, acc[:],
                                                   ALU.is_gt, ALU.add)
            for j in range(n_act):
                e = n_dve + j
                bias = negxm[:, off + e : off + e + 1]
                k = j % NA
                if j < NA:
                    nc.scalar.activation(accA[k][:], sb_ix[:],
                                         mybir.ActivationFunctionType.Sign,
                                         bias=bias, scale=1.0)
                else:
                    s = sgn[j % 8]
                    nc.scalar.activation(s[:], sb_ix[:],
                                         mybir.ActivationFunctionType.Sign,
                                         bias=bias, scale=1.0)
                    nc.gpsimd.tensor_tensor(accA[k][:], accA[k][:], s[:], ALU.add)
            # Pool clamp lane: c = min(max(ix - theta, 0), 1) with theta the
            # half-integer -negxm => c == [x > xm] exactly
            for j in range(n_pool):
                e = n_dve + n_act + j
                col = negxm[:, off + e : off + e + 1]
                s = clp[j % 4]
                nc.gpsimd.tensor_scalar(s[:], sb_ix[:], col, 0.5, ALU.add, ALU.add)
                k = j % NP
                if j < NP:
                    nc.gpsimd.tensor_scalar(accP[k][:], s[:], 0.0, 1.0, ALU.max,
                                            ALU.min)
                else:
                    s2 = clp[j % 4]
                    nc.gpsimd.tensor_scalar(s2[:], s[:], 0.0, 1.0, ALU.max, ALU.min)
                    nc.gpsimd.tensor_tensor(accP[k][:], accP[k][:], s2[:], ALU.add)

            cm1 = accp.tile([128, XN], F32, tag="cm1")
            nc.gpsimd.tensor_tensor(cm1[:], accD[0][:], accD[1][:], ALU.add)
            cm2 = accp.tile([128, XN], F32, tag="cm2")
            nc.gpsimd.tensor_tensor(cm2[:], accD[2][:], accD[3][:], ALU.add)
            cm2b = accp.tile([128, XN], F32, tag="cm2b")
            nc.gpsimd.tensor_tensor(cm2b[:], cm2[:], accD[4][:], ALU.add)
            cm3 = accp.tile([128, XN], F32, tag="cm3")
            nc.gpsimd.tensor_tensor(cm3[:], accA[0][:], accA[1][:], ALU.add)
            cm4 = accp.tile([128, XN], F32, tag="cm4")
            nc.gpsimd.tensor_tensor(cm4[:], accA[2][:], accA[3][:], ALU.add)
            cm5 = accp.tile([128, XN], F32, tag="cm5")
            nc.gpsimd.tensor_tensor(cm5[:], cm1[:], cm2b[:], ALU.add)
            cm6 = accp.tile([128, XN], F32, tag="cm6")
            nc.gpsimd.tensor_tensor(cm6[:], cm3[:], cm4[:], ALU.add)
            cm7 = accp.tile([128, XN], F32, tag="cm7")
            nc.gpsimd.tensor_scalar(cm7[:], cm6[:], float(NV - EDGES_DVE - EDGES_POOL),
                                    0.5, ALU.add, ALU.mult)
            cm8 = accp.tile([128, XN], F32, tag="cm8")
            nc.gpsimd.tensor_tensor(cm8[:], accP[0][:], accP[1][:], ALU.add)
            cm9 = accp.tile([128, XN], F32, tag="cm9")
            nc.gpsimd.tensor_tensor(cm9[:], cm5[:], cm8[:], ALU.add)
            accT = accp.tile([128, XN], F32, tag="accT")
            nc.gpsimd.tensor_tensor(accT[:], cm9[:], cm7[:], ALU.add)

            # mask = (A mod 2) | (A(x+1) > A(x));   dice partial sums
            # parity(A) exactly in f32: r = rnd_half_even(A/2) via the 2^23
            # trick, d = A - 2r in {0, +-1}, par = d^2
            TWO23 = 8388608.0
            u = post.tile([128, GRID], F32, tag="paru")
            nc.gpsimd.tensor_scalar(u[:], accT[:, 0:GRID], 0.5, TWO23,
                                    ALU.mult, ALU.add)
            r = post.tile([128, GRID], F32, tag="parr")
            nc.gpsimd.tensor_scalar(r[:], u[:], -TWO23, None, ALU.add)
            dpar = post.tile([128, GRID], F32, tag="pard")
            nc.vector.scalar_tensor_tensor(dpar[:], r[:], -2.0, accT[:, 0:GRID],
                                           ALU.mult, ALU.add)
            par = post.tile([128, GRID], F32, tag="par")
            nc.gpsimd.tensor_tensor(par[:], dpar[:], dpar[:], ALU.mult)
            bnd = post.tile([128, GRID], F32, tag="bnd")
            nc.vector.tensor_tensor(bnd[:], accT[:, 1:XN], accT[:, 0:GRID], ALU.is_gt)
            col0 = 6 * b + 3 * c
            mask = post.tile([128, GRID], F32, tag="mask")
            nc.vector.scalar_tensor_tensor(
                mask[:], par[:], 0.0, bnd[:], ALU.add, ALU.max,
                accum_out=sb_stats[:, col0 : col0 + 1])

            # q = (dmap*255 <= 127), rewritten as dmap <= Q_THRESH (exact:
            # x -> fl(x*255) is monotone, Q_THRESH is the largest f32 passing).
            # op1 here is the accumulator's reduce op (sum -> Q stat).
            q = post.tile([128, GRID], F32, tag="q")
            nc.vector.tensor_scalar(q[:], sb_dm[:], Q_THRESH, None, ALU.is_le,
                                    ALU.add,
                                    accum_out=sb_stats[:, col0 + 2 : col0 + 3])

            prod = post.tile([128, GRID], F32, tag="prod")
            nc.vector.scalar_tensor_tensor(
                prod[:], mask[:], 0.0, q[:], ALU.add, ALU.mult,
                accum_out=sb_stats[:, col0 + 1 : col0 + 2])

    # ---------------- final reduction over partitions ----------------
    ps_stats = psum.tile([6 * B, 1], F32)
    nc.tensor.matmul(ps_stats[:], sb_stats[:], sb_onescol[:],
                     start=True, stop=True)
    sb_final = setup.tile([6 * B, 1], F32)
    nc.vector.tensor_copy(sb_final[:], ps_stats[:])
    nc.sync.dma_start(stats_d[:], sb_final[:])


def _build():
    if "nc" in _CACHE:
        return _CACHE["nc"]
    nc = bacc.Bacc(None, target_bir_lowering=False, debug=False)
    pts_d = nc.dram_tensor("pts", [NV, 2 * B], F32, kind="ExternalInput")
    dmap_d = nc.dram_tensor("dmap", [B, GRID, GRID], F32, kind="ExternalInput")
    ident_d = nc.dram_tensor("ident", [128, 128], F32, kind="ExternalInput")
    stats_d = nc.dram_tensor("stats", [6 * B, 1], F32, kind="ExternalOutput")
    from contextlib import ExitStack

    with tile.TileContext(nc) as tc:
        with ExitStack() as ctx:
            _emit(ctx, tc, pts_d, dmap_d, ident_d, stats_d)
    if hasattr(nc, "compile"):
        nc.compile()
    else:
        nc.finalize()
    _CACHE["nc"] = nc
    return nc


def kernel(points: np.ndarray, dmap: np.ndarray) -> np.ndarray:
    pts = np.asarray(points, dtype=np.float32).reshape(64, NV, 2)
    dm = np.asarray(dmap, dtype=np.float32).reshape(64, GRID, GRID)
    ident = np.eye(128, dtype=np.float32)

    in_maps = []
    for r in range(N_CORES):
        sl = slice(r * B, (r + 1) * B)
        pts_r = np.ascontiguousarray(pts[sl].transpose(1, 0, 2).reshape(NV, 2 * B))
        in_maps.append({
            "pts": pts_r,
            "dmap": np.ascontiguousarray(dm[sl]),
            "ident": ident,
        })

    nc = _build()
    res = run_bass_kernel_spmd(nc, in_maps, core_ids=list(range(N_CORES)))

    dices = []
    for r in range(N_CORES):
        s = np.asarray(res.results[r]["stats"], dtype=np.float32).reshape(B, 2, 3)
        s = s.sum(axis=1)  # combine the two row-chunks
        T, I, Q = s[:, 0], s[:, 1], s[:, 2]
        dices.append((np.float32(2.0) * I + np.float32(SMOOTH))
                     / (T + Q + np.float32(SMOOTH)))
    dices = np.concatenate(dices).astype(np.float32)
    return np.float32(np.mean(np.float32(1.0) - dices))
